# revision 8
# baseline (speedup 1.0000x reference)
"""Trainium2 Bass kernel v2 for nn_Attention_54589034332712.

Sharding: 8 cores = 4 batches x 2 head-halves (tensor parallel over heads,
per the sharding hint).  Core c handles batch c//2 and heads
[8*(c%2), 8*(c%2)+8) for all 1024 queries.  Each core computes a partial
output projection over its 8 heads; the halves are summed at gather time
(device collectives fail to load in this environment, so the all-reduce of
the hint happens host-side as part of unsharding).

Mask specialization (exact, derived from the actual mask values at build
time, so any 0/1 mask is handled correctly):
  The reference computes w*mask - finfo.min*(1-mask): masked entries get a
  huge positive bias, so for any query row with >=1 masked entry softmax
  underflows the unmasked weights to exactly 0 and distributes uniformly
  over masked entries.  We compute P_num = exp(scores) + BT where
  BT = C*(1-maskT), C = 2^115.  For q-chunks where ALL rows have >=1 masked
  entry, P_num = BT alone is exact (unmasked weights are exactly 0 in the
  reference), so scores/exp are skipped and the AV matmul consumes BT
  directly.  Blocks with no masked entries skip the BT add.  Denominators
  come free from a ones column appended to V; division uses fp32 reciprocal
  + a rank-1 f32r broadcast matmul.  All matmuls in float32r.
"""

import sys

sys.path.insert(0, "/opt/trn_rl_repo")

import os

import numpy as np

import concourse.bacc as bacc
import concourse.bass as bass
import concourse.mybir as mybir
import concourse.tile as tile
from concourse.bass_utils import run_bass_kernel_spmd

f32 = mybir.dt.float32
f32r = mybir.dt.float32r
u32 = mybir.dt.uint32
Act = mybir.ActivationFunctionType
Alu = mybir.AluOpType

B, S, E, H = 4, 1024, 1024, 16
D = E // H  # 64
HH = H // 2  # heads per core (8)
NG = HH // 2  # local head groups of 2 (4)
EC = E // 128  # contraction chunks (8)
KC = S // 128  # k chunks (8)
QC = S // 512  # q chunks (2)
MASK_C = float(2.0**115)
N_CORES = 8
ONE_F32_BITS = 1065353216

SC_BUFS = int(os.environ.get("KSC_BUFS", "3"))
EP_BUFS = int(os.environ.get("KEP_BUFS", "3"))
MM_BUFS = int(os.environ.get("KMM_BUFS", "2"))

_program_cache = {}


def classify_mask(attn_mask):
    """Per q-chunk execution mode + per-block mask info, uniform across cores.

    Modes per 512-row q-chunk:
      ("degen", None): every row has >=1 masked entry -> P_num = BT exactly
        (reference softmax underflows unmasked weights to exactly 0).
      ("corr", (r0, r1)): like degen except a small contiguous range of rows
        [r0, r1) has no masked entries; those columns get a dense-softmax
        correction accumulated into the AV psum.
      ("full", None): general path (scores+exp for every block, BT add where
        the block has masked entries).
    """
    m = np.asarray(attn_mask) != 0.0  # True = keep
    row_has_masked = ~m.all(axis=1)  # (S,)
    modes = []
    block_has_masked = []
    for qc in range(QC):
        rows = slice(512 * qc, 512 * (qc + 1))
        rhm = row_has_masked[rows]
        live = np.nonzero(~rhm)[0]
        if len(live) == 0:
            modes.append(("degen", None))
        elif len(live) <= 64 and live[-1] - live[0] + 1 == len(live):
            # f32r matmuls need even moving sizes and 8B-aligned starts; pad
            # the range into degenerate rows (their e^S contributions are
            # exactly absorbed by the 2^115 mask terms).
            r0 = int(live[0]) & ~1
            r1 = int(live[-1]) + 1
            w = r1 - r0
            w += w % 2
            if r0 + w > 512:
                r0 = 512 - w
            modes.append(("corr", (r0, r0 + w)))
        else:
            modes.append(("full", None))
        block_has_masked.append(
            tuple(
                bool((~m[rows, 128 * j : 128 * (j + 1)]).any()) for j in range(KC)
            )
        )
    return tuple(modes), tuple(block_has_masked)


def build_program(qc_modes, block_has_masked):
    key = (qc_modes, block_has_masked)
    if key in _program_cache:
        return _program_cache[key]
    nc = bacc.Bacc("TRN2", target_bir_lowering=False, debug=False, num_devices=N_CORES)

    hT_d = nc.dram_tensor("hT", [E, S], f32, kind="ExternalInput").ap()
    maskT_d = nc.dram_tensor("maskT", [S, S], f32, kind="ExternalInput").ap()
    wqkv_d = nc.dram_tensor("w_qkv_half", [E, 3 * 512], f32, kind="ExternalInput").ap()
    wp_d = nc.dram_tensor("w_proj_half", [512, E], f32, kind="ExternalInput").ap()
    bqkv_d = nc.dram_tensor("b_qkv_half", [3 * 512], f32, kind="ExternalInput").ap()
    bproj_d = nc.dram_tensor("b_proj_in", [E], f32, kind="ExternalInput").ap()
    out_d = nc.dram_tensor("out", [S, E], f32, kind="ExternalOutput").ap()

    # BT slots needed: for degenerate chunks every j; for live chunks only
    # blocks with masked entries.
    bt_slots = {}
    for qc in range(QC):
        for j in range(KC):
            if qc_modes[qc][0] in ("degen", "corr") or block_has_masked[qc][j]:
                bt_slots[(qc, j)] = len(bt_slots)
    n_bt = max(1, len(bt_slots))

    with tile.TileContext(nc) as tc:
        with (
            tc.tile_pool(name="const", bufs=1) as constp,
            tc.tile_pool(name="qt", bufs=1) as qtp,
            tc.tile_pool(name="kt", bufs=1) as ktp,
            tc.tile_pool(name="vv", bufs=1) as vvp,
            tc.tile_pool(name="bt", bufs=1) as btp,
            tc.tile_pool(name="avall", bufs=1) as avallp,
        ):
            ones_f = constp.tile([1, 128], f32)
            nc.vector.memset(ones_f[:], 1.0)
            ones = constp.tile([1, 128], f32r)
            nc.vector.tensor_copy(ones[:], ones_f[:])
            onescol_f = constp.tile([128, 1], f32)
            nc.vector.memset(onescol_f[:], 1.0)
            ones_col = constp.tile([128, 1], f32r)
            nc.vector.tensor_copy(ones_col[:], onescol_f[:])
            cbias = constp.tile([128, 1], f32)
            nc.vector.memset(cbias[:], MASK_C)

            bqkv_sb = constp.tile([128, 8], f32)  # q,k biases as columns
            nc.sync.dma_start(
                bqkv_sb[:], bqkv_d[0:1024].rearrange("(c p) -> p c", p=128)
            )
            bq_s = constp.tile([128, 4], f32)
            nc.scalar.mul(bq_s[:], bqkv_sb[:, 0:4], 0.125)

            bv0 = constp.tile([1, 512], f32r)
            nc.sync.dma_start(
                bv0[:],
                bqkv_d[1024:1536].rearrange("(c t) -> c t", c=1).bitcast(f32r),
            )
            bp0 = constp.tile([1, 512], f32r)
            bp1 = constp.tile([1, 512], f32r)
            nc.sync.dma_start(
                bp0[:], bproj_d[0:512].rearrange("(c t) -> c t", c=1).bitcast(f32r)
            )
            nc.sync.dma_start(
                bp1[:], bproj_d[512:E].rearrange("(c t) -> c t", c=1).bitcast(f32r)
            )

            QT = qtp.tile([128, NG * S], f32r)
            KT = ktp.tile([128, NG * S], f32r)
            V = vvp.tile([128, KC * 512], f32r)  # plain: chunk t, head h at 512t+64h
            BT = btp.tile([128, n_bt * 512], f32r)
            av_all = avallp.tile([128, NG * S], f32r)

            # --- phase A: load + QKV ---
            with (
                tc.tile_pool(name="ht", bufs=1) as htp,
                tc.tile_pool(name="mstage", bufs=1) as msp,
                tc.tile_pool(name="wqk", bufs=2) as wqkp,
                tc.tile_pool(name="wvp", bufs=1) as wvp,
                tc.tile_pool(name="mm", bufs=MM_BUFS, space="PSUM") as mmps,
            ):
                hT = htp.tile([128, EC * S], f32r)
                nc.sync.dma_start(
                    hT[:].rearrange("p (c t) -> p c t", t=S),
                    hT_d.bitcast(f32r).rearrange("(c p) t -> p c t", p=128),
                )
                mstage = msp.tile([128, KC * S], f32)
                nc.sync.dma_start(
                    mstage[:].rearrange("p (c q) -> p c q", q=S),
                    maskT_d.rearrange("(c p) q -> p c q", p=128),
                )
                for (qc, j), slot in bt_slots.items():
                    nc.scalar.activation(
                        BT[:, 512 * slot : 512 * (slot + 1)],
                        mstage[:, S * j + 512 * qc : S * j + 512 * (qc + 1)],
                        Act.Identity,
                        bias=cbias[:],
                        scale=-MASK_C,
                    )

                for g in range(NG):
                    wq = wqkp.tile([128, EC * 128], f32r, tag="wq")
                    wk = wqkp.tile([128, EC * 128], f32r, tag="wk")
                    nc.sync.dma_start(
                        wq[:].rearrange("p (c d) -> p c d", d=128),
                        wqkv_d[:, 128 * g : 128 * (g + 1)]
                        .bitcast(f32r)
                        .rearrange("(c p) d -> p c d", p=128),
                    )
                    nc.sync.dma_start(
                        wk[:].rearrange("p (c d) -> p c d", d=128),
                        wqkv_d[:, 512 + 128 * g : 512 + 128 * (g + 1)]
                        .bitcast(f32r)
                        .rearrange("(c p) d -> p c d", p=128),
                    )
                    for t in range(QC):
                        ps = mmps.tile([128, 512], f32, tag="mm")
                        for e in range(EC):
                            nc.tensor.matmul(
                                ps[:],
                                wq[:, 128 * e : 128 * (e + 1)],
                                hT[:, S * e + 512 * t : S * e + 512 * (t + 1)],
                                start=(e == 0),
                                stop=(e == EC - 1),
                            )
                        nc.scalar.activation(
                            QT[:, S * g + 512 * t : S * g + 512 * (t + 1)],
                            ps[:],
                            Act.Identity,
                            bias=bq_s[:, g : g + 1],
                            scale=0.125,
                        )
                        ps2 = mmps.tile([128, 512], f32, tag="mm")
                        for e in range(EC):
                            nc.tensor.matmul(
                                ps2[:],
                                wk[:, 128 * e : 128 * (e + 1)],
                                hT[:, S * e + 512 * t : S * e + 512 * (t + 1)],
                                start=(e == 0),
                                stop=(e == EC - 1),
                            )
                        nc.scalar.activation(
                            KT[:, S * g + 512 * t : S * g + 512 * (t + 1)],
                            ps2[:],
                            Act.Identity,
                            bias=bqkv_sb[:, 4 + g : 5 + g],
                            scale=1.0,
                        )

                wv = wvp.tile([128, EC * 512], f32r)
                nc.sync.dma_start(
                    wv[:].rearrange("p (cc d) -> p cc d", d=512),
                    wqkv_d[:, 1024:1536]
                    .bitcast(f32r)
                    .rearrange("(cc p) d -> p cc d", p=128),
                )
                for t in range(KC):
                    ps3 = mmps.tile([128, 512], f32, tag="mm")
                    for e in range(EC):
                        nc.tensor.matmul(
                            ps3[:],
                            hT[:, S * e + 128 * t : S * e + 128 * (t + 1)],
                            wv[:, 512 * e : 512 * (e + 1)],
                            start=(e == 0),
                            stop=False,
                        )
                    nc.tensor.matmul(
                        ps3[:], ones[0:1, 0:128], bv0[0:1, :], start=False, stop=True
                    )
                    v_out = V[:, 520 * t : 520 * (t + 1)].rearrange(
                        "p (i dd) -> p i dd", dd=65
                    )[:, :, 0:64]
                    nc.vector.tensor_copy(
                        v_out, ps3[:].rearrange("p (i d) -> p i d", d=64)
                    )
                v_ones = V[:].rearrange("p (t i dd) -> p t i dd", i=HH, dd=65)[
                    :, :, :, 64:65
                ]
                nc.gpsimd.memset(v_ones.bitcast(u32), ONE_F32_BITS)

            # --- prefetch w_proj during attention ---
            wpp_cm = tc.tile_pool(name="wp", bufs=1)
            wpp = wpp_cm.__enter__()
            wp_t = []
            for g in range(NG):
                w = wpp.tile([128, E], f32r, tag=f"wp{g}", name=f"wp_{g}")
                nc.sync.dma_start(
                    w[:], wp_d[128 * g : 128 * (g + 1), :].bitcast(f32r)
                )
                wp_t.append(w)

            # --- phase B: attention ---
            with (
                tc.tile_pool(name="sc", bufs=SC_BUFS, space="PSUM") as scps,
                tc.tile_pool(name="avps", bufs=2, space="PSUM") as avps,
                tc.tile_pool(name="bc", bufs=1, space="PSUM") as bcps,
                tc.tile_pool(name="ee", bufs=EP_BUFS) as eep,
                tc.tile_pool(name="pp", bufs=EP_BUFS) as ppp,
                tc.tile_pool(name="avtmp", bufs=2) as avtp,
                tc.tile_pool(name="rc", bufs=1) as rcp,
            ):
                recips = rcp.tile([1, HH * QC * 512], f32r)
                btden_sb = rcp.tile([1, QC * 512], f32r)
                btdraw_sb = rcp.tile([1, QC * 512], f32)
                bcast_sb = rcp.tile([128, QC * 512], f32)
                # shared denominators for BT-direct chunks: Sum_k BT[k, q]
                for qc in range(QC):
                    mode, rng = qc_modes[qc]
                    if mode == "full":
                        continue
                    btd = bcps.tile([1, 512], f32, tag="small", name=f"btd_{qc}")
                    for j in range(KC):
                        nc.tensor.matmul(
                            btd[:],
                            ones_col[:],
                            BT[:, 512 * bt_slots[(qc, j)] : 512 * (bt_slots[(qc, j)] + 1)],
                            start=(j == 0),
                            stop=(j == KC - 1),
                        )
                    nc.scalar.copy(btdraw_sb[0:1, 512 * qc : 512 * (qc + 1)], btd[:])
                    with nc.allow_low_precision(reason="f32r recip for bcast"):
                        nc.vector.reciprocal(
                            btden_sb[0:1, 512 * qc : 512 * (qc + 1)], btd[:]
                        )
                    bcq = bcps.tile([128, 512], f32, tag="bc", name=f"bcq_{qc}")
                    nc.tensor.matmul(
                        bcq[:],
                        ones[0:1, 0:128],
                        btden_sb[0:1, 512 * qc : 512 * (qc + 1)],
                        start=True,
                        stop=True,
                    )
                    nc.scalar.copy(bcast_sb[:, 512 * qc : 512 * (qc + 1)], bcq[:])

                for g in range(NG):
                    for qc in range(QC):
                        mode, rng = qc_modes[qc]
                        if mode in ("degen", "corr"):
                            av2 = avps.tile([128, 512], f32, tag="av", name=f"av2_{g}_{qc}")
                            for j in range(KC):
                                nc.tensor.matmul(
                                    av2[:],
                                    V[:, 512 * j + 128 * g : 512 * j + 128 * (g + 1)],
                                    BT[
                                        :,
                                        512 * bt_slots[(qc, j)] : 512
                                        * (bt_slots[(qc, j)] + 1),
                                    ],
                                    start=(j == 0),
                                    stop=(j == KC - 1),
                                )
                            cds_s = []
                            cps_s = []
                            if mode == "corr":
                                r0, r1 = rng
                                w = r1 - r0
                                for s in range(2):
                                    sc_col = scps.tile(
                                        [128, KC * w], f32, tag="sc",
                                        name=f"scc{g}{qc}{s}",
                                    )
                                    for j in range(KC):
                                        nc.tensor.matmul(
                                            sc_col[:, j * w : (j + 1) * w],
                                            KT[
                                                64 * s : 64 * (s + 1),
                                                S * g + 128 * j : S * g + 128 * (j + 1),
                                            ],
                                            QT[
                                                64 * s : 64 * (s + 1),
                                                S * g + 512 * qc + r0 : S * g
                                                + 512 * qc
                                                + r1,
                                            ],
                                            start=True,
                                            stop=True,
                                            skip_group_check=True,
                                        )
                                    eec = eep.tile(
                                        [128, KC * w], f32r, tag="eec",
                                        name=f"eec{g}{qc}{s}",
                                    )
                                    nc.scalar.activation(eec[:], sc_col[:], Act.Exp)
                                    cps = avps.tile(
                                        [64, w], f32, tag="cps", name=f"cps{g}{qc}{s}"
                                    )
                                    for j in range(KC):
                                        nc.tensor.matmul(
                                            cps[:],
                                            V[
                                                :,
                                                512 * j + 128 * g + 64 * s : 512 * j
                                                + 128 * g
                                                + 64 * (s + 1),
                                            ],
                                            eec[:, j * w : (j + 1) * w],
                                            start=(j == 0),
                                            stop=(j == KC - 1),
                                        )
                                    cps_s.append(cps)
                                    # per-head denominator delta for corr cols
                                    cd = bcps.tile(
                                        [1, w * KC], f32, tag="small", name=f"cd{g}{qc}{s}"
                                    )
                                    nc.tensor.matmul(
                                        cd[:],
                                        ones_col[:],
                                        eec[:].rearrange("p (j wi) -> p wi j", wi=w),
                                        start=True,
                                        stop=True,
                                    )
                                    cds = avtp.tile(
                                        [1, w], f32, tag=f"cds{s}", name=f"cds{g}{qc}{s}"
                                    )
                                    nc.vector.tensor_reduce(
                                        cds[:],
                                        cd[:].rearrange("p (wi j) -> p wi j", wi=w),
                                        axis=mybir.AxisListType.X,
                                        op=Alu.add,
                                    )
                                    cds_s.append(cds)
                            # divide (writes garbage into corr cols; fixed below)
                            nc.vector.tensor_tensor(
                                out=av_all[:, S * g + 512 * qc : S * g + 512 * (qc + 1)],
                                in0=av2[:],
                                in1=bcast_sb[:, 512 * qc : 512 * (qc + 1)],
                                op=Alu.mult,
                            )
                            if mode == "corr":
                                r0, r1 = rng
                                w = r1 - r0
                                for s in range(2):
                                    # combined numerator: BT part (av2) + e^S
                                    # part (cps); fp32 add swallows exactly the
                                    # right one on both degenerate-padded and
                                    # live rows.
                                    cps_sb = avtp.tile(
                                        [64, w], f32, tag="cpssb", name=f"cb{g}{qc}{s}"
                                    )
                                    nc.scalar.copy(cps_sb[:], cps_s[s][:])
                                    val_sb = avtp.tile(
                                        [64, w], f32, tag="valsb", name=f"vl{g}{qc}{s}"
                                    )
                                    nc.vector.tensor_tensor(
                                        out=val_sb[:],
                                        in0=av2[64 * s : 64 * (s + 1), r0:r1],
                                        in1=cps_sb[:],
                                        op=Alu.add,
                                    )
                                    dcomb = avtp.tile(
                                        [1, w], f32, tag="dcomb", name=f"dc{g}{qc}{s}"
                                    )
                                    nc.vector.tensor_tensor(
                                        out=dcomb[:],
                                        in0=cds_s[s][:],
                                        in1=btdraw_sb[
                                            0:1,
                                            512 * qc + r0 : 512 * qc + r1,
                                        ],
                                        op=Alu.add,
                                    )
                                    rcw = avtp.tile(
                                        [1, w], f32r, tag="rcw", name=f"rcw{g}{qc}{s}"
                                    )
                                    with nc.allow_low_precision(reason="recip"):
                                        nc.vector.reciprocal(rcw[:], dcomb[:])
                                    bcw = bcps.tile(
                                        [64, w], f32, tag="bc", name=f"bcw{g}{qc}{s}"
                                    )
                                    nc.tensor.matmul(
                                        bcw[:], ones[0:1, 0:64], rcw[:],
                                        start=True, stop=True,
                                    )
                                    bcw_sb = avtp.tile(
                                        [64, w], f32, tag="bcwsb", name=f"bw{g}{qc}{s}"
                                    )
                                    nc.scalar.copy(bcw_sb[:], bcw[:])
                                    nc.vector.tensor_tensor(
                                        out=av_all[
                                            64 * s : 64 * (s + 1),
                                            S * g + 512 * qc + r0 : S * g
                                            + 512 * qc
                                            + r1,
                                        ],
                                        in0=val_sb[:],
                                        in1=bcw_sb[:],
                                        op=Alu.mult,
                                    )
                        else:
                            # full path: per-head scores/exp/(BT add)/AV + denom
                            # (s passes deinterleaved so one dn slot suffices)
                            av_t = [
                                avps.tile([64, 512], f32, tag="av", name=f"avf_{g}_{qc}_{s}")
                                for s in range(2)
                            ]
                            for s in range(2):
                                dn = bcps.tile(
                                    [1, 512], f32, tag="dn", name=f"dn_{g}_{qc}_{s}"
                                )
                                for j in range(KC):
                                    sc = scps.tile(
                                        [128, 512], f32, tag="sc", name=f"sc{g}{qc}{j}{s}"
                                    )
                                    nc.tensor.matmul(
                                        sc[:],
                                        KT[
                                            64 * s : 64 * (s + 1),
                                            S * g + 128 * j : S * g + 128 * (j + 1),
                                        ],
                                        QT[
                                            64 * s : 64 * (s + 1),
                                            S * g + 512 * qc : S * g + 512 * (qc + 1),
                                        ],
                                        start=True,
                                        stop=True,
                                    )
                                    ee = eep.tile(
                                        [128, 512], f32r, tag="ee", name=f"ee{g}{qc}{j}{s}"
                                    )
                                    nc.scalar.activation(ee[:], sc[:], Act.Exp)
                                    if block_has_masked[qc][j]:
                                        pp = ppp.tile(
                                            [128, 512], f32r, tag="pp",
                                            name=f"pp{g}{qc}{j}{s}",
                                        )
                                        nc.vector.tensor_tensor(
                                            out=pp[:],
                                            in0=ee[:],
                                            in1=BT[
                                                :,
                                                512 * bt_slots[(qc, j)] : 512
                                                * (bt_slots[(qc, j)] + 1),
                                            ],
                                            op=Alu.add,
                                        )
                                        rhs = pp[:]
                                    else:
                                        rhs = ee[:]
                                    nc.tensor.matmul(
                                        av_t[s][:],
                                        V[
                                            :,
                                            512 * j + 128 * g + 64 * s : 512 * j
                                            + 128 * g
                                            + 64 * (s + 1),
                                        ],
                                        rhs,
                                        start=(j == 0),
                                        stop=(j == KC - 1),
                                    )
                                    nc.tensor.matmul(
                                        dn[:],
                                        ones_col[:],
                                        rhs,
                                        start=(j == 0),
                                        stop=(j == KC - 1),
                                        skip_group_check=True,
                                    )
                                hq = (2 * g + s) * QC + qc
                                with nc.allow_low_precision(reason="recip"):
                                    nc.vector.reciprocal(
                                        recips[0:1, 512 * hq : 512 * (hq + 1)],
                                        dn[:],
                                    )
                            for s in range(2):
                                hq = (2 * g + s) * QC + qc
                                bc = bcps.tile(
                                    [64, 512], f32, tag="bc", name=f"bcf{g}{qc}{s}"
                                )
                                nc.tensor.matmul(
                                    bc[:],
                                    ones[0:1, 0:64],
                                    recips[0:1, 512 * hq : 512 * (hq + 1)],
                                    start=True,
                                    stop=True,
                                )
                                avt = avtp.tile(
                                    [64, 512], f32, tag="avt", name=f"avtf{g}{qc}{s}"
                                )
                                nc.vector.tensor_copy(avt[:], av_t[s][:])
                                nc.vector.tensor_tensor(
                                    out=av_all[
                                        64 * s : 64 * (s + 1),
                                        S * g + 512 * qc : S * g + 512 * (qc + 1),
                                    ],
                                    in0=avt[:],
                                    in1=bc[:],
                                    op=Alu.mult,
                                )

            # --- phase C: partial output projection ---
            with (
                tc.tile_pool(name="outp", bufs=2) as outp,
                tc.tile_pool(name="mm2", bufs=2, space="PSUM") as mmps2,
            ):
                for t in range(KC):
                    out_sb = outp.tile([128, E], f32, tag="out", name=f"out_{t}")
                    for c in range(2):
                        ps = mmps2.tile([128, 512], f32, tag="mm2", name=f"pj_{t}_{c}")
                        for g in range(NG):
                            nc.tensor.matmul(
                                ps[:],
                                av_all[:, S * g + 128 * t : S * g + 128 * (t + 1)],
                                wp_t[g][:, 512 * c : 512 * (c + 1)],
                                start=(g == 0),
                                stop=False,
                            )
                        nc.tensor.matmul(
                            ps[:],
                            ones[0:1, 0:128],
                            (bp0 if c == 0 else bp1)[0:1, :],
                            start=False,
                            stop=True,
                        )
                        nc.vector.tensor_copy(out_sb[:, 512 * c : 512 * (c + 1)], ps[:])
                        nc.sync.dma_start(
                            out_d[128 * t : 128 * (t + 1), 512 * c : 512 * (c + 1)],
                            out_sb[:, 512 * c : 512 * (c + 1)],
                        )
            wpp_cm.__exit__(None, None, None)

    nc.compile()
    _program_cache[key] = nc
    return nc


def kernel(hidden_states, w_qkv, b_qkv, w_proj, b_proj, attn_mask):
    hidden_states = np.ascontiguousarray(np.asarray(hidden_states, dtype=np.float32))
    w_qkv = np.ascontiguousarray(np.asarray(w_qkv, dtype=np.float32))
    b_qkv = np.ascontiguousarray(np.asarray(b_qkv, dtype=np.float32))
    w_proj = np.ascontiguousarray(np.asarray(w_proj, dtype=np.float32))
    b_proj = np.ascontiguousarray(np.asarray(b_proj, dtype=np.float32))
    attn_mask = np.ascontiguousarray(np.asarray(attn_mask, dtype=np.float32))

    maskT = np.ascontiguousarray(attn_mask.T)
    zeros_bp = np.zeros_like(b_proj)
    in_maps = []
    for c in range(N_CORES):
        b, hh = c // 2, c % 2
        cols = slice(512 * hh, 512 * (hh + 1))
        w_half = np.ascontiguousarray(
            np.concatenate(
                [w_qkv[:, cols], w_qkv[:, E + 512 * hh : E + 512 * (hh + 1)],
                 w_qkv[:, 2 * E + 512 * hh : 2 * E + 512 * (hh + 1)]],
                axis=1,
            )
        )
        b_half = np.ascontiguousarray(
            np.concatenate(
                [b_qkv[cols], b_qkv[E + 512 * hh : E + 512 * (hh + 1)],
                 b_qkv[2 * E + 512 * hh : 2 * E + 512 * (hh + 1)]]
            )
        )
        in_maps.append(
            {
                "hT": np.ascontiguousarray(hidden_states[b].T),
                "maskT": maskT,
                "w_qkv_half": w_half,
                "w_proj_half": np.ascontiguousarray(w_proj[cols, :]),
                "b_qkv_half": b_half,
                "b_proj_in": b_proj if hh == 0 else zeros_bp,
            }
        )

    qc_modes, blk = classify_mask(attn_mask)
    nc = build_program(qc_modes, blk)
    res = run_bass_kernel_spmd(nc, in_maps, core_ids=list(range(N_CORES)))

    out = np.empty((B, S, E), dtype=np.float32)
    for b in range(B):
        out[b] = res.results[2 * b]["out"] + res.results[2 * b + 1]["out"]
    return out


if __name__ == "__main__":
    rng = np.random.default_rng(0)
    inputs = {
        "hidden_states": rng.standard_normal((B, S, E)).astype(np.float32),
        "w_qkv": (rng.standard_normal((E, 3 * E)) * 0.02).astype(np.float32),
        "b_qkv": np.zeros(3 * E, np.float32),
        "w_proj": (rng.standard_normal((E, E)) * 0.02).astype(np.float32),
        "b_proj": np.zeros(E, np.float32),
        "attn_mask": np.tril(np.ones((S, S), np.float32)),
    }
    out = kernel(**inputs)
    print("kernel ran, out shape", out.shape, "finite:", np.isfinite(out).all())


# revision 10
# speedup vs baseline: 1.1255x; 1.1255x over previous
"""Trainium2 Bass kernel v2 for nn_Attention_54589034332712.

Sharding: 8 cores = 4 batches x 2 head-halves (tensor parallel over heads,
per the sharding hint).  Core c handles batch c//2 and heads
[8*(c%2), 8*(c%2)+8) for all 1024 queries.  Each core computes a partial
output projection over its 8 heads; the halves are summed at gather time
(device collectives fail to load in this environment, so the all-reduce of
the hint happens host-side as part of unsharding).

Mask specialization (exact, derived from the actual mask values at build
time, so any 0/1 mask is handled correctly):
  The reference computes w*mask - finfo.min*(1-mask): masked entries get a
  huge positive bias, so for any query row with >=1 masked entry softmax
  underflows the unmasked weights to exactly 0 and distributes uniformly
  over masked entries.  We compute P_num = exp(scores) + BT where
  BT = C*(1-maskT), C = 2^115.  For q-chunks where ALL rows have >=1 masked
  entry, P_num = BT alone is exact (unmasked weights are exactly 0 in the
  reference), so scores/exp are skipped and the AV matmul consumes BT
  directly.  Blocks with no masked entries skip the BT add.  Denominators
  come free from a ones column appended to V; division uses fp32 reciprocal
  + a rank-1 f32r broadcast matmul.  All matmuls in float32r.
"""

import sys

sys.path.insert(0, "/opt/trn_rl_repo")

import os

import numpy as np

import concourse.bacc as bacc
import concourse.bass as bass
import concourse.mybir as mybir
import concourse.tile as tile
from concourse.bass_utils import run_bass_kernel_spmd

f32 = mybir.dt.float32
f32r = mybir.dt.float32r
u32 = mybir.dt.uint32
Act = mybir.ActivationFunctionType
Alu = mybir.AluOpType

B, S, E, H = 4, 1024, 1024, 16
D = E // H  # 64
HH = H // 2  # heads per core (8)
NG = HH // 2  # local head groups of 2 (4)
EC = E // 128  # contraction chunks (8)
KC = S // 128  # k chunks (8)
QC = S // 512  # q chunks (2)
MASK_C = float(2.0**115)
N_CORES = 8
ONE_F32_BITS = 1065353216

SC_BUFS = int(os.environ.get("KSC_BUFS", "3"))
EP_BUFS = int(os.environ.get("KEP_BUFS", "3"))
MM_BUFS = int(os.environ.get("KMM_BUFS", "2"))

_program_cache = {}


def classify_mask(attn_mask):
    """Per q-chunk execution mode + per-block mask info, uniform across cores.

    Modes per 512-row q-chunk:
      ("degen", None): every row has >=1 masked entry -> P_num = BT exactly
        (reference softmax underflows unmasked weights to exactly 0).
      ("corr", (r0, r1)): like degen except a small contiguous range of rows
        [r0, r1) has no masked entries; those columns get a dense-softmax
        correction accumulated into the AV psum.
      ("full", None): general path (scores+exp for every block, BT add where
        the block has masked entries).
    """
    m = np.asarray(attn_mask) != 0.0  # True = keep
    row_has_masked = ~m.all(axis=1)  # (S,)
    modes = []
    block_has_masked = []
    for qc in range(QC):
        rows = slice(512 * qc, 512 * (qc + 1))
        rhm = row_has_masked[rows]
        live = np.nonzero(~rhm)[0]
        if len(live) == 0:
            modes.append(("degen", None))
        elif len(live) <= 64 and live[-1] - live[0] + 1 == len(live):
            # f32r matmuls need even moving sizes and 8B-aligned starts; pad
            # the range into degenerate rows (their e^S contributions are
            # exactly absorbed by the 2^115 mask terms).
            r0 = int(live[0]) & ~1
            r1 = int(live[-1]) + 1
            w = r1 - r0
            w += w % 2
            if r0 + w > 512:
                r0 = 512 - w
            modes.append(("corr", (r0, r0 + w)))
        else:
            modes.append(("full", None))
        block_has_masked.append(
            tuple(
                bool((~m[rows, 128 * j : 128 * (j + 1)]).any()) for j in range(KC)
            )
        )
    return tuple(modes), tuple(block_has_masked)


def build_program(qc_modes, block_has_masked):
    key = (qc_modes, block_has_masked)
    if key in _program_cache:
        return _program_cache[key]
    nc = bacc.Bacc("TRN2", target_bir_lowering=False, debug=False, num_devices=N_CORES)

    hT_d = nc.dram_tensor("hT", [E, S], f32, kind="ExternalInput").ap()
    maskT_d = nc.dram_tensor("maskT", [S, S], mybir.dt.uint8, kind="ExternalInput").ap()
    wqkv_d = nc.dram_tensor("w_qkv_half", [E, 3 * 512], f32, kind="ExternalInput").ap()
    wp_d = nc.dram_tensor("w_proj_half", [512, E], f32, kind="ExternalInput").ap()
    bqkv_d = nc.dram_tensor("b_qkv_half", [3 * 512], f32, kind="ExternalInput").ap()
    bproj_d = nc.dram_tensor("b_proj_in", [E], f32, kind="ExternalInput").ap()
    out_d = nc.dram_tensor("out", [S, E], f32, kind="ExternalOutput").ap()

    # BT slots needed: for degenerate chunks every j; for live chunks only
    # blocks with masked entries.
    bt_slots = {}
    for qc in range(QC):
        for j in range(KC):
            if qc_modes[qc][0] in ("degen", "corr") or block_has_masked[qc][j]:
                bt_slots[(qc, j)] = len(bt_slots)
    n_bt = max(1, len(bt_slots))

    with tile.TileContext(nc) as tc:
        with (
            tc.tile_pool(name="const", bufs=1) as constp,
            tc.tile_pool(name="qt", bufs=1) as qtp,
            tc.tile_pool(name="kt", bufs=1) as ktp,
            tc.tile_pool(name="vv", bufs=1) as vvp,
            tc.tile_pool(name="bt", bufs=1) as btp,
            tc.tile_pool(name="avall", bufs=1) as avallp,
        ):
            ones_f = constp.tile([1, 128], f32)
            nc.vector.memset(ones_f[:], 1.0)
            ones = constp.tile([1, 128], f32r)
            nc.vector.tensor_copy(ones[:], ones_f[:])
            onescol_f = constp.tile([128, 1], f32)
            nc.vector.memset(onescol_f[:], 1.0)
            ones_col = constp.tile([128, 1], f32r)
            nc.vector.tensor_copy(ones_col[:], onescol_f[:])
            cbias = constp.tile([128, 1], f32)
            nc.vector.memset(cbias[:], MASK_C)

            bqkv_sb = constp.tile([128, 8], f32)  # q,k biases as columns
            nc.sync.dma_start(
                bqkv_sb[:], bqkv_d[0:1024].rearrange("(c p) -> p c", p=128)
            )
            bq_s = constp.tile([128, 4], f32)
            nc.scalar.mul(bq_s[:], bqkv_sb[:, 0:4], 0.125)

            bv0 = constp.tile([1, 512], f32r)
            nc.sync.dma_start(
                bv0[:],
                bqkv_d[1024:1536].rearrange("(c t) -> c t", c=1).bitcast(f32r),
            )
            bp0 = constp.tile([1, 512], f32r)
            bp1 = constp.tile([1, 512], f32r)
            nc.sync.dma_start(
                bp0[:], bproj_d[0:512].rearrange("(c t) -> c t", c=1).bitcast(f32r)
            )
            nc.sync.dma_start(
                bp1[:], bproj_d[512:E].rearrange("(c t) -> c t", c=1).bitcast(f32r)
            )

            QT = qtp.tile([128, NG * S], f32r)
            KT = ktp.tile([128, NG * S], f32r)
            V = vvp.tile([128, KC * 512], f32r)  # plain: chunk t, head h at 512t+64h
            BT = btp.tile([128, n_bt * 512], f32r)
            av_all = avallp.tile([128, NG * S], f32r)

            # --- phase A: load + QKV ---
            with (
                tc.tile_pool(name="ht", bufs=1) as htp,
                tc.tile_pool(name="mstage", bufs=1) as msp,
                tc.tile_pool(name="wqk", bufs=2) as wqkp,
                tc.tile_pool(name="wvp", bufs=1) as wvp,
                tc.tile_pool(name="mm", bufs=MM_BUFS, space="PSUM") as mmps,
            ):
                hT = htp.tile([128, EC * S], f32r)
                nc.sync.dma_start(
                    hT[:].rearrange("p (c t) -> p c t", t=S),
                    hT_d.bitcast(f32r).rearrange("(c p) t -> p c t", p=128),
                )
                mstage = msp.tile([128, KC * S], f32)
                nc.sync.dma_start(
                    mstage[:].rearrange("p (c q) -> p c q", q=S),
                    maskT_d.rearrange("(c p) q -> p c q", p=128),
                )
                for (qc, j), slot in bt_slots.items():
                    nc.scalar.activation(
                        BT[:, 512 * slot : 512 * (slot + 1)],
                        mstage[:, S * j + 512 * qc : S * j + 512 * (qc + 1)],
                        Act.Identity,
                        bias=cbias[:],
                        scale=-MASK_C,
                    )

                for g in range(NG):
                    wq = wqkp.tile([128, EC * 128], f32r, tag="wq")
                    wk = wqkp.tile([128, EC * 128], f32r, tag="wk")
                    nc.sync.dma_start(
                        wq[:].rearrange("p (c d) -> p c d", d=128),
                        wqkv_d[:, 128 * g : 128 * (g + 1)]
                        .bitcast(f32r)
                        .rearrange("(c p) d -> p c d", p=128),
                    )
                    nc.sync.dma_start(
                        wk[:].rearrange("p (c d) -> p c d", d=128),
                        wqkv_d[:, 512 + 128 * g : 512 + 128 * (g + 1)]
                        .bitcast(f32r)
                        .rearrange("(c p) d -> p c d", p=128),
                    )
                    for t in range(QC):
                        ps = mmps.tile([128, 512], f32, tag="mm")
                        for e in range(EC):
                            nc.tensor.matmul(
                                ps[:],
                                wq[:, 128 * e : 128 * (e + 1)],
                                hT[:, S * e + 512 * t : S * e + 512 * (t + 1)],
                                start=(e == 0),
                                stop=(e == EC - 1),
                            )
                        nc.scalar.activation(
                            QT[:, S * g + 512 * t : S * g + 512 * (t + 1)],
                            ps[:],
                            Act.Identity,
                            bias=bq_s[:, g : g + 1],
                            scale=0.125,
                        )
                        ps2 = mmps.tile([128, 512], f32, tag="mm")
                        for e in range(EC):
                            nc.tensor.matmul(
                                ps2[:],
                                wk[:, 128 * e : 128 * (e + 1)],
                                hT[:, S * e + 512 * t : S * e + 512 * (t + 1)],
                                start=(e == 0),
                                stop=(e == EC - 1),
                            )
                        nc.scalar.activation(
                            KT[:, S * g + 512 * t : S * g + 512 * (t + 1)],
                            ps2[:],
                            Act.Identity,
                            bias=bqkv_sb[:, 4 + g : 5 + g],
                            scale=1.0,
                        )

                wv = wvp.tile([128, EC * 512], f32r)
                nc.sync.dma_start(
                    wv[:].rearrange("p (cc d) -> p cc d", d=512),
                    wqkv_d[:, 1024:1536]
                    .bitcast(f32r)
                    .rearrange("(cc p) d -> p cc d", p=128),
                )
                for t in range(KC):
                    ps3 = mmps.tile([128, 512], f32, tag="mm")
                    for e in range(EC):
                        nc.tensor.matmul(
                            ps3[:],
                            hT[:, S * e + 128 * t : S * e + 128 * (t + 1)],
                            wv[:, 512 * e : 512 * (e + 1)],
                            start=(e == 0),
                            stop=False,
                        )
                    nc.tensor.matmul(
                        ps3[:], ones[0:1, 0:128], bv0[0:1, :], start=False, stop=True
                    )
                    v_out = V[:, 520 * t : 520 * (t + 1)].rearrange(
                        "p (i dd) -> p i dd", dd=65
                    )[:, :, 0:64]
                    nc.vector.tensor_copy(
                        v_out, ps3[:].rearrange("p (i d) -> p i d", d=64)
                    )
                v_ones = V[:].rearrange("p (t i dd) -> p t i dd", i=HH, dd=65)[
                    :, :, :, 64:65
                ]
                nc.gpsimd.memset(v_ones.bitcast(u32), ONE_F32_BITS)

            # --- prefetch w_proj during attention ---
            wpp_cm = tc.tile_pool(name="wp", bufs=1)
            wpp = wpp_cm.__enter__()
            wp_t = []
            for g in range(NG):
                w = wpp.tile([128, E], f32r, tag=f"wp{g}", name=f"wp_{g}")
                nc.sync.dma_start(
                    w[:], wp_d[128 * g : 128 * (g + 1), :].bitcast(f32r)
                )
                wp_t.append(w)

            # --- phase B: attention ---
            with (
                tc.tile_pool(name="sc", bufs=SC_BUFS, space="PSUM") as scps,
                tc.tile_pool(name="avps", bufs=2, space="PSUM") as avps,
                tc.tile_pool(name="bc", bufs=1, space="PSUM") as bcps,
                tc.tile_pool(name="ee", bufs=EP_BUFS) as eep,
                tc.tile_pool(name="pp", bufs=EP_BUFS) as ppp,
                tc.tile_pool(name="avtmp", bufs=2) as avtp,
                tc.tile_pool(name="rc", bufs=1) as rcp,
            ):
                recips = rcp.tile([1, HH * QC * 512], f32r)
                btden_sb = rcp.tile([1, QC * 512], f32r)
                btdraw_sb = rcp.tile([1, QC * 512], f32)
                bcast_sb = rcp.tile([128, QC * 512], f32)
                # shared denominators for BT-direct chunks: Sum_k BT[k, q]
                for qc in range(QC):
                    mode, rng = qc_modes[qc]
                    if mode == "full":
                        continue
                    btd = bcps.tile([1, 512], f32, tag="small", name=f"btd_{qc}")
                    for j in range(KC):
                        nc.tensor.matmul(
                            btd[:],
                            ones_col[:],
                            BT[:, 512 * bt_slots[(qc, j)] : 512 * (bt_slots[(qc, j)] + 1)],
                            start=(j == 0),
                            stop=(j == KC - 1),
                        )
                    nc.scalar.copy(btdraw_sb[0:1, 512 * qc : 512 * (qc + 1)], btd[:])
                    with nc.allow_low_precision(reason="f32r recip for bcast"):
                        nc.vector.reciprocal(
                            btden_sb[0:1, 512 * qc : 512 * (qc + 1)], btd[:]
                        )
                    bcq = bcps.tile([128, 512], f32, tag="bc", name=f"bcq_{qc}")
                    nc.tensor.matmul(
                        bcq[:],
                        ones[0:1, 0:128],
                        btden_sb[0:1, 512 * qc : 512 * (qc + 1)],
                        start=True,
                        stop=True,
                    )
                    nc.scalar.copy(bcast_sb[:, 512 * qc : 512 * (qc + 1)], bcq[:])

                for g in range(NG):
                    for qc in range(QC):
                        mode, rng = qc_modes[qc]
                        if mode in ("degen", "corr"):
                            av2 = avps.tile([128, 512], f32, tag="av", name=f"av2_{g}_{qc}")
                            for j in range(KC):
                                nc.tensor.matmul(
                                    av2[:],
                                    V[:, 512 * j + 128 * g : 512 * j + 128 * (g + 1)],
                                    BT[
                                        :,
                                        512 * bt_slots[(qc, j)] : 512
                                        * (bt_slots[(qc, j)] + 1),
                                    ],
                                    start=(j == 0),
                                    stop=(j == KC - 1),
                                )
                            cds_s = []
                            cps_s = []
                            if mode == "corr":
                                r0, r1 = rng
                                w = r1 - r0
                                for s in range(2):
                                    sc_col = scps.tile(
                                        [128, KC * w], f32, tag="sc",
                                        name=f"scc{g}{qc}{s}",
                                    )
                                    for j in range(KC):
                                        nc.tensor.matmul(
                                            sc_col[:, j * w : (j + 1) * w],
                                            KT[
                                                64 * s : 64 * (s + 1),
                                                S * g + 128 * j : S * g + 128 * (j + 1),
                                            ],
                                            QT[
                                                64 * s : 64 * (s + 1),
                                                S * g + 512 * qc + r0 : S * g
                                                + 512 * qc
                                                + r1,
                                            ],
                                            start=True,
                                            stop=True,
                                            skip_group_check=True,
                                        )
                                    eec = eep.tile(
                                        [128, KC * w], f32r, tag="eec",
                                        name=f"eec{g}{qc}{s}",
                                    )
                                    nc.scalar.activation(eec[:], sc_col[:], Act.Exp)
                                    cps = avps.tile(
                                        [64, w], f32, tag="cps", name=f"cps{g}{qc}{s}"
                                    )
                                    for j in range(KC):
                                        nc.tensor.matmul(
                                            cps[:],
                                            V[
                                                :,
                                                512 * j + 128 * g + 64 * s : 512 * j
                                                + 128 * g
                                                + 64 * (s + 1),
                                            ],
                                            eec[:, j * w : (j + 1) * w],
                                            start=(j == 0),
                                            stop=(j == KC - 1),
                                        )
                                    cps_s.append(cps)
                                    # per-head denominator delta for corr cols
                                    cd = bcps.tile(
                                        [1, w * KC], f32, tag="small", name=f"cd{g}{qc}{s}"
                                    )
                                    nc.tensor.matmul(
                                        cd[:],
                                        ones_col[:],
                                        eec[:].rearrange("p (j wi) -> p wi j", wi=w),
                                        start=True,
                                        stop=True,
                                    )
                                    cds = avtp.tile(
                                        [1, w], f32, tag=f"cds{s}", name=f"cds{g}{qc}{s}"
                                    )
                                    nc.vector.tensor_reduce(
                                        cds[:],
                                        cd[:].rearrange("p (wi j) -> p wi j", wi=w),
                                        axis=mybir.AxisListType.X,
                                        op=Alu.add,
                                    )
                                    cds_s.append(cds)
                            # divide (writes garbage into corr cols; fixed below)
                            nc.vector.tensor_tensor(
                                out=av_all[:, S * g + 512 * qc : S * g + 512 * (qc + 1)],
                                in0=av2[:],
                                in1=bcast_sb[:, 512 * qc : 512 * (qc + 1)],
                                op=Alu.mult,
                            )
                            if mode == "corr":
                                r0, r1 = rng
                                w = r1 - r0
                                for s in range(2):
                                    # combined numerator: BT part (av2) + e^S
                                    # part (cps); fp32 add swallows exactly the
                                    # right one on both degenerate-padded and
                                    # live rows.
                                    cps_sb = avtp.tile(
                                        [64, w], f32, tag="cpssb", name=f"cb{g}{qc}{s}"
                                    )
                                    nc.scalar.copy(cps_sb[:], cps_s[s][:])
                                    val_sb = avtp.tile(
                                        [64, w], f32, tag="valsb", name=f"vl{g}{qc}{s}"
                                    )
                                    nc.vector.tensor_tensor(
                                        out=val_sb[:],
                                        in0=av2[64 * s : 64 * (s + 1), r0:r1],
                                        in1=cps_sb[:],
                                        op=Alu.add,
                                    )
                                    dcomb = avtp.tile(
                                        [1, w], f32, tag="dcomb", name=f"dc{g}{qc}{s}"
                                    )
                                    nc.vector.tensor_tensor(
                                        out=dcomb[:],
                                        in0=cds_s[s][:],
                                        in1=btdraw_sb[
                                            0:1,
                                            512 * qc + r0 : 512 * qc + r1,
                                        ],
                                        op=Alu.add,
                                    )
                                    rcw = avtp.tile(
                                        [1, w], f32r, tag="rcw", name=f"rcw{g}{qc}{s}"
                                    )
                                    with nc.allow_low_precision(reason="recip"):
                                        nc.vector.reciprocal(rcw[:], dcomb[:])
                                    bcw = bcps.tile(
                                        [64, w], f32, tag="bc", name=f"bcw{g}{qc}{s}"
                                    )
                                    nc.tensor.matmul(
                                        bcw[:], ones[0:1, 0:64], rcw[:],
                                        start=True, stop=True,
                                    )
                                    bcw_sb = avtp.tile(
                                        [64, w], f32, tag="bcwsb", name=f"bw{g}{qc}{s}"
                                    )
                                    nc.scalar.copy(bcw_sb[:], bcw[:])
                                    nc.vector.tensor_tensor(
                                        out=av_all[
                                            64 * s : 64 * (s + 1),
                                            S * g + 512 * qc + r0 : S * g
                                            + 512 * qc
                                            + r1,
                                        ],
                                        in0=val_sb[:],
                                        in1=bcw_sb[:],
                                        op=Alu.mult,
                                    )
                        else:
                            # full path: per-head scores/exp/(BT add)/AV + denom
                            # (s passes deinterleaved so one dn slot suffices)
                            av_t = [
                                avps.tile([64, 512], f32, tag="av", name=f"avf_{g}_{qc}_{s}")
                                for s in range(2)
                            ]
                            for s in range(2):
                                dn = bcps.tile(
                                    [1, 512], f32, tag="dn", name=f"dn_{g}_{qc}_{s}"
                                )
                                for j in range(KC):
                                    sc = scps.tile(
                                        [128, 512], f32, tag="sc", name=f"sc{g}{qc}{j}{s}"
                                    )
                                    nc.tensor.matmul(
                                        sc[:],
                                        KT[
                                            64 * s : 64 * (s + 1),
                                            S * g + 128 * j : S * g + 128 * (j + 1),
                                        ],
                                        QT[
                                            64 * s : 64 * (s + 1),
                                            S * g + 512 * qc : S * g + 512 * (qc + 1),
                                        ],
                                        start=True,
                                        stop=True,
                                    )
                                    ee = eep.tile(
                                        [128, 512], f32r, tag="ee", name=f"ee{g}{qc}{j}{s}"
                                    )
                                    nc.scalar.activation(ee[:], sc[:], Act.Exp)
                                    if block_has_masked[qc][j]:
                                        pp = ppp.tile(
                                            [128, 512], f32r, tag="pp",
                                            name=f"pp{g}{qc}{j}{s}",
                                        )
                                        nc.vector.tensor_tensor(
                                            out=pp[:],
                                            in0=ee[:],
                                            in1=BT[
                                                :,
                                                512 * bt_slots[(qc, j)] : 512
                                                * (bt_slots[(qc, j)] + 1),
                                            ],
                                            op=Alu.add,
                                        )
                                        rhs = pp[:]
                                    else:
                                        rhs = ee[:]
                                    nc.tensor.matmul(
                                        av_t[s][:],
                                        V[
                                            :,
                                            512 * j + 128 * g + 64 * s : 512 * j
                                            + 128 * g
                                            + 64 * (s + 1),
                                        ],
                                        rhs,
                                        start=(j == 0),
                                        stop=(j == KC - 1),
                                    )
                                    nc.tensor.matmul(
                                        dn[:],
                                        ones_col[:],
                                        rhs,
                                        start=(j == 0),
                                        stop=(j == KC - 1),
                                        skip_group_check=True,
                                    )
                                hq = (2 * g + s) * QC + qc
                                with nc.allow_low_precision(reason="recip"):
                                    nc.vector.reciprocal(
                                        recips[0:1, 512 * hq : 512 * (hq + 1)],
                                        dn[:],
                                    )
                            for s in range(2):
                                hq = (2 * g + s) * QC + qc
                                bc = bcps.tile(
                                    [64, 512], f32, tag="bc", name=f"bcf{g}{qc}{s}"
                                )
                                nc.tensor.matmul(
                                    bc[:],
                                    ones[0:1, 0:64],
                                    recips[0:1, 512 * hq : 512 * (hq + 1)],
                                    start=True,
                                    stop=True,
                                )
                                avt = avtp.tile(
                                    [64, 512], f32, tag="avt", name=f"avtf{g}{qc}{s}"
                                )
                                nc.vector.tensor_copy(avt[:], av_t[s][:])
                                nc.vector.tensor_tensor(
                                    out=av_all[
                                        64 * s : 64 * (s + 1),
                                        S * g + 512 * qc : S * g + 512 * (qc + 1),
                                    ],
                                    in0=avt[:],
                                    in1=bc[:],
                                    op=Alu.mult,
                                )

            # --- phase C: partial output projection ---
            with (
                tc.tile_pool(name="outp", bufs=2) as outp,
                tc.tile_pool(name="mm2", bufs=2, space="PSUM") as mmps2,
            ):
                for t in range(KC):
                    out_sb = outp.tile([128, E], f32, tag="out", name=f"out_{t}")
                    for c in range(2):
                        ps = mmps2.tile([128, 512], f32, tag="mm2", name=f"pj_{t}_{c}")
                        for g in range(NG):
                            nc.tensor.matmul(
                                ps[:],
                                av_all[:, S * g + 128 * t : S * g + 128 * (t + 1)],
                                wp_t[g][:, 512 * c : 512 * (c + 1)],
                                start=(g == 0),
                                stop=False,
                            )
                        nc.tensor.matmul(
                            ps[:],
                            ones[0:1, 0:128],
                            (bp0 if c == 0 else bp1)[0:1, :],
                            start=False,
                            stop=True,
                        )
                        nc.vector.tensor_copy(out_sb[:, 512 * c : 512 * (c + 1)], ps[:])
                        nc.sync.dma_start(
                            out_d[128 * t : 128 * (t + 1), 512 * c : 512 * (c + 1)],
                            out_sb[:, 512 * c : 512 * (c + 1)],
                        )
            wpp_cm.__exit__(None, None, None)

    nc.compile()
    _program_cache[key] = nc
    return nc


def kernel(hidden_states, w_qkv, b_qkv, w_proj, b_proj, attn_mask):
    hidden_states = np.ascontiguousarray(np.asarray(hidden_states, dtype=np.float32))
    w_qkv = np.ascontiguousarray(np.asarray(w_qkv, dtype=np.float32))
    b_qkv = np.ascontiguousarray(np.asarray(b_qkv, dtype=np.float32))
    w_proj = np.ascontiguousarray(np.asarray(w_proj, dtype=np.float32))
    b_proj = np.ascontiguousarray(np.asarray(b_proj, dtype=np.float32))
    attn_mask = np.ascontiguousarray(np.asarray(attn_mask, dtype=np.float32))

    maskT_u8 = np.ascontiguousarray((attn_mask.T != 0.0).astype(np.uint8))
    zeros_bp = np.zeros_like(b_proj)
    in_maps = []
    for c in range(N_CORES):
        b, hh = c // 2, c % 2
        cols = slice(512 * hh, 512 * (hh + 1))
        w_half = np.ascontiguousarray(
            np.concatenate(
                [w_qkv[:, cols], w_qkv[:, E + 512 * hh : E + 512 * (hh + 1)],
                 w_qkv[:, 2 * E + 512 * hh : 2 * E + 512 * (hh + 1)]],
                axis=1,
            )
        )
        b_half = np.ascontiguousarray(
            np.concatenate(
                [b_qkv[cols], b_qkv[E + 512 * hh : E + 512 * (hh + 1)],
                 b_qkv[2 * E + 512 * hh : 2 * E + 512 * (hh + 1)]]
            )
        )
        in_maps.append(
            {
                "hT": np.ascontiguousarray(hidden_states[b].T),
                "maskT": maskT_u8,
                "w_qkv_half": w_half,
                "w_proj_half": np.ascontiguousarray(w_proj[cols, :]),
                "b_qkv_half": b_half,
                "b_proj_in": b_proj if hh == 0 else zeros_bp,
            }
        )

    qc_modes, blk = classify_mask(attn_mask)
    nc = build_program(qc_modes, blk)
    res = run_bass_kernel_spmd(nc, in_maps, core_ids=list(range(N_CORES)))

    out = np.empty((B, S, E), dtype=np.float32)
    for b in range(B):
        out[b] = res.results[2 * b]["out"] + res.results[2 * b + 1]["out"]
    return out


if __name__ == "__main__":
    rng = np.random.default_rng(0)
    inputs = {
        "hidden_states": rng.standard_normal((B, S, E)).astype(np.float32),
        "w_qkv": (rng.standard_normal((E, 3 * E)) * 0.02).astype(np.float32),
        "b_qkv": np.zeros(3 * E, np.float32),
        "w_proj": (rng.standard_normal((E, E)) * 0.02).astype(np.float32),
        "b_proj": np.zeros(E, np.float32),
        "attn_mask": np.tril(np.ones((S, S), np.float32)),
    }
    out = kernel(**inputs)
    print("kernel ran, out shape", out.shape, "finite:", np.isfinite(out).all())
